# revision 41
# baseline (speedup 1.0000x reference)
"""GCN (3x GCNConv + global max pool + FC + log_softmax) on 8 Trainium2 NeuronCores.

Strategy (v3, continuous packing + chunk-major scheduling):
  - 1D partition of nodes: core c owns rows [12500c, 12500(c+1)).
  - Table for layer l holds hs_l = dinv * (a_l @ W_l) rows (bf16, 128-wide,
    256B gather elems), split into 4 chunks (source quarters), each the
    AllGather (quarter-interleaved across cores) of one quarter of every
    core's rows.
  - Aggregation per 128-dst tile via gpsimd dma_gather + PE one-hot sum.
    v3: slots are packed CONTINUOUSLY within each (wave, chunk) gather
    call (tiles back-to-back, one 128-roundup per call) instead of
    rounding every (tile, chunk) group to 128-slot blocks. Tile segments
    may straddle msg blocks; each (tile, overlapped-block) pair gets its
    own host-built one-hot sel matrix (per-core data), so the static
    instruction stream is core-independent while per-core edge counts
    differ. Cuts gather descriptors (the gpsimd bottleneck) ~14%.
  - v3 scheduling: AllGather triggers (which run on gpsimd and WAIT for
    their source bounces) are interleaved with gather emission so the
    in-order gpsimd queue never head-of-line blocks: at layer starts the
    ch<3 gathers of the first waves are emitted chunk-major before the
    first ch3 gather, and each AG trigger is emitted only after the wave
    whose PE work already covers its source quarter.
  - Self-loops never gathered: identity-matmul from the SBUF-resident
    local hs tile; bias via a K=1 outer-product matmul.
  - Post per tile fused: scalar relu(dinv * psum) -> bf16, PE transpose,
    scalar copy, next layer's dense matmul + dinv scale immediately.
  - Pooling (segment max over graphs), tiny FC and log_softmax on host.
"""

import os
import sys

sys.path.insert(0, "/opt/trn_rl_repo")

import numpy as np
import ml_dtypes

import concourse.bass as bass
import concourse.bacc as bacc
import concourse.tile as tile
from concourse import mybir
from concourse.bass_utils import run_bass_kernel_spmd

P = 128
N_NODES = 100000
N_EDGES = 1600000
N_GRAPHS = 64
N_CORES = 8
NPC = N_NODES // N_CORES                     # 12500
NTILES = (NPC + P - 1) // P                  # 98 (last tile 84 rows)
LAST_ROWS = NPC - (NTILES - 1) * P           # 84
F0 = 512
FW = 128                                     # table row width (256B gather elem)
NCH = 4                                      # chunks = src quarters of each core
QT = [0, 25, 50, 75, 98]                     # quarter tile boundaries
QSTART = [0, 3200, 6400, 9600]               # quarter row starts
QROWS = [3200, 3200, 3200, 2900]             # rows per quarter per core
SQR = [3200, 3200, 3200, 2944]               # 128-aligned per-core chunk stride
CH_ROWS = [N_CORES * r for r in SQR]         # table rows per chunk
NCOLS = [128, 64, 32]                        # real table width per layer
WT = int(os.environ.get("GCN_WT", "7"))      # target tiles per wave
MAXB = int(os.environ.get("GCN_MAXB", "34"))  # max blocks per dma_gather call
NQUEUES = 4
DMA_SCRATCH = int(os.environ.get("GCN_SCRATCH", "30720"))
SEL_F8 = os.environ.get("GCN_SELDT", "f8") == "f8"
TBL_SHARED = os.environ.get("GCN_SHARED", "1") == "1"
MSG_BUFS = int(os.environ.get("GCN_MSGB", "13"))
SEL_BUFS = int(os.environ.get("GCN_SELB", "2"))
TRIG_LAG = int(os.environ.get("GCN_TRIGLAG", "3"))  # waves of PE lag before AG trigger
dt = mybir.dt
BF = ml_dtypes.bfloat16
F8 = ml_dtypes.float8_e4m3
SEL_DT = dt.float8e4 if SEL_F8 else dt.bfloat16
SEL_NP = F8 if SEL_F8 else BF


def _rows(t):
    return LAST_ROWS if t == NTILES - 1 else P


def _quarter_of_tile(t):
    for q in range(4):
        if t < QT[q + 1]:
            return q
    raise AssertionError


def _make_waves():
    waves = []
    for q in range(4):
        tiles = list(range(QT[q], QT[q + 1]))
        nw = (len(tiles) + WT - 1) // WT
        for part in np.array_split(np.array(tiles), nw):
            waves.append((q, [int(t) for t in part]))
    return waves


def _host_prep(edge_index):
    """Shared (cross-core) aggregation schedule + per-core index/sel data."""
    src = edge_index[0].astype(np.int64)
    dst = edge_index[1].astype(np.int64)
    deg = np.bincount(dst, minlength=N_NODES).astype(np.float64) + 1.0
    dinv = (1.0 / np.sqrt(deg)).astype(np.float32)

    waves = _make_waves()
    NW = len(waves)
    wave_of_tile = np.zeros(NTILES, np.int64)
    for wi, (q, ts) in enumerate(waves):
        for t in ts:
            wave_of_tile[t] = wi

    # chunk position of a src node: quarter-interleaved table layout
    s_core = src // NPC
    s_loc = src % NPC
    s_tile = s_loc // P
    s_q = np.digitize(s_tile, QT[1:4])              # 0..3
    qrows = np.array(SQR)[s_q]
    qstart = np.array(QSTART)[s_q]
    s_pos = s_core * qrows + (s_loc - qstart)       # position within chunk s_q

    core_of = dst // NPC

    # per-core counts per (wave, chunk, tile)
    cnt3 = np.zeros((N_CORES, NW, NCH, NTILES), np.int64)
    per_core_raw = []
    for c in range(N_CORES):
        m = core_of == c
        pos, chv = s_pos[m], s_q[m]
        dl = dst[m] - c * NPC
        t = dl // P
        w = wave_of_tile[t]
        key = (w * NCH + chv) * NTILES + t
        o = np.argsort(key, kind="stable")
        pos, chv, t, dl, w, key = pos[o], chv[o], t[o], dl[o], w[o], key[o]
        cnt3[c] = np.bincount(key, minlength=NW * NCH * NTILES).reshape(
            NW, NCH, NTILES
        )
        per_core_raw.append((pos, chv, t, dl - t * P, w, key))

    # within-call offsets of each tile segment (per core), call sizes
    off_tc = np.zeros((N_CORES, NTILES, NCH), np.int64)
    cnt_tc = np.zeros((N_CORES, NTILES, NCH), np.int64)
    cnt_call = np.zeros((N_CORES, NW, NCH), np.int64)
    for c in range(N_CORES):
        for wi, (q, ts) in enumerate(waves):
            for ch in range(NCH):
                acc = 0
                for t in ts:
                    off_tc[c, t, ch] = acc
                    cnt_tc[c, t, ch] = cnt3[c, wi, ch, t]
                    acc += cnt3[c, wi, ch, t]
                cnt_call[c, wi, ch] = acc
    call_nb = np.maximum((cnt_call.max(axis=0) + P - 1) // P, 1)  # [NW, NCH]

    # static per-(tile, chunk) block span (union over cores)
    blk0 = np.zeros((NTILES, NCH), np.int64)
    blk1 = np.zeros((NTILES, NCH), np.int64)
    any_cnt = cnt_tc.max(axis=0) > 0                # [NTILES, NCH]
    for t in range(NTILES):
        for ch in range(NCH):
            if not any_cnt[t, ch]:
                continue
            s = off_tc[:, t, ch]
            e = s + cnt_tc[:, t, ch]
            act = cnt_tc[:, t, ch] > 0
            blk0[t, ch] = (s[act] // P).min()
            blk1[t, ch] = ((e[act] + P - 1) // P).max()

    # sel order: (wave, tile, chunk, block)
    sel_base = np.zeros((NTILES, NCH), np.int64)
    wave_sel_start = np.zeros(NW + 1, np.int64)
    g = 0
    for wi, (q, ts) in enumerate(waves):
        wave_sel_start[wi] = g
        for t in ts:
            for ch in range(NCH):
                if not any_cnt[t, ch]:
                    continue
                sel_base[t, ch] = g
                g += int(blk1[t, ch] - blk0[t, ch])
    wave_sel_start[NW] = g
    NB_SEL = g

    # idx slot order per chunk: (wave, block)
    chunk_start = np.zeros((NCH, NW), np.int64)
    S_ch = [0] * NCH
    for ch in range(NCH):
        acc = 0
        for wi in range(NW):
            chunk_start[ch, wi] = acc
            acc += int(call_nb[wi, ch]) * P
        S_ch[ch] = acc

    idx16 = [np.zeros((N_CORES, P, S_ch[ch] // 16), np.int16) for ch in range(NCH)]
    sel = np.zeros((N_CORES, P, NB_SEL, P), SEL_NP)

    for c in range(N_CORES):
        pos, chv, t, dd, w, key = per_core_raw[c]
        flat = np.bincount(key, minlength=NW * NCH * NTILES)
        starts = np.cumsum(flat) - flat
        i = np.arange(len(pos)) - starts[key]       # seq within (w, ch, t)
        slot_in_call = off_tc[c, t, chv] + i
        j = slot_in_call // P
        lane = slot_in_call % P
        selpos = sel_base[t, chv] + (j - blk0[t, chv])
        sel[c][lane, selpos, dd] = 1.0
        chunk_slot = chunk_start[chv, w] + slot_in_call
        for ch in range(NCH):
            mm = chv == ch
            slots_arr = np.zeros(S_ch[ch], np.int16)
            slots_arr[chunk_slot[mm]] = pos[mm].astype(np.int16)
            idx16[ch][c] = np.tile(slots_arr.reshape(-1, 16).T, (8, 1))

    meta = {
        "waves": waves,
        "call_nb": call_nb,
        "blk0": blk0,
        "blk1": blk1,
        "any_cnt": any_cnt,
        "sel_base": sel_base,
        "wave_sel_start": wave_sel_start,
        "NB_SEL": NB_SEL,
        "chunk_start": chunk_start,
        "S_ch": S_ch,
    }
    return dinv, idx16, sel, meta


def _build_program(meta):
    waves = meta["waves"]
    call_nb = meta["call_nb"]
    blk0 = meta["blk0"]
    blk1 = meta["blk1"]
    any_cnt = meta["any_cnt"]
    sel_base = meta["sel_base"]
    wave_sel_start = meta["wave_sel_start"]
    NB_SEL = meta["NB_SEL"]
    chunk_start = meta["chunk_start"]
    S_ch = meta["S_ch"]
    NW = len(waves)

    MSG_NB = int(call_nb.max())
    SEL_NB = int(np.diff(wave_sel_start).max())

    nc = bacc.Bacc(
        "TRN2", target_bir_lowering=False, debug=False, num_devices=N_CORES,
        num_swdge_queues=NQUEUES, dynamic_dma_scratch_size=DMA_SCRATCH,
    )

    xT_io = nc.dram_tensor("xT", [F0, NPC], dt.bfloat16, kind="ExternalInput").ap()
    dinv_io = nc.dram_tensor("dinvT", [P, NTILES], dt.float32, kind="ExternalInput").ap()
    rdinv_io = nc.dram_tensor("rdinvR", [65, 6272], dt.bfloat16, kind="ExternalInput").ap()
    w1_io = nc.dram_tensor("W1sb", [P, F0], dt.bfloat16, kind="ExternalInput").ap()
    w2_io = nc.dram_tensor("W2sb", [P, 64], dt.bfloat16, kind="ExternalInput").ap()
    w3_io = nc.dram_tensor("W3sb", [64, 32], dt.bfloat16, kind="ExternalInput").ap()
    brow_ios = [
        nc.dram_tensor(f"b{l+1}row", [65, NCOLS[l]], dt.bfloat16, kind="ExternalInput").ap()
        for l in range(3)
    ]
    identb_io = nc.dram_tensor("identB", [P, P], dt.float32, kind="ExternalInput").ap()
    identf8_io = nc.dram_tensor("identF8", [P, P], SEL_DT, kind="ExternalInput").ap()
    identbb_io = nc.dram_tensor("identBB", [P, P], dt.bfloat16, kind="ExternalInput").ap()
    idx_ios = [
        nc.dram_tensor(f"idx{ch}", [P, S_ch[ch] // 16], dt.int16, kind="ExternalInput").ap()
        for ch in range(NCH)
    ]
    sel_io = nc.dram_tensor("sel", [P, NB_SEL * P], SEL_DT, kind="ExternalInput").ap()
    out_io = nc.dram_tensor("out3", [NPC, 32], dt.float32, kind="ExternalOutput").ap()

    with tile.TileContext(nc) as tc:
        with (
            tc.tile_pool(name="const", bufs=1) as constp,
            tc.tile_pool(name="hskeep", bufs=1) as hkp,
            tc.tile_pool(name="xT", bufs=4) as xTp,
            tc.tile_pool(name="idxw", bufs=7) as idxp,
            tc.tile_pool(name="msgs", bufs=MSG_BUFS) as msgp,
            tc.tile_pool(name="sel", bufs=SEL_BUFS) as selp,
            tc.tile_pool(name="outt", bufs=7) as outtp,
            tc.tile_pool(name="att", bufs=3) as attp,
            tc.tile_pool(name="psum_a", bufs=4, space="PSUM") as psap,
            tc.tile_pool(name="psum_t", bufs=2, space="PSUM") as pstp,
            tc.tile_pool(name="psum_d", bufs=2, space="PSUM") as psdp,
            tc.tile_pool(name="dram", bufs=1, space="DRAM") as dramp,
        ):
            # ---- constants ----
            dinv_sb = constp.tile([P, NTILES], dt.float32)
            nc.sync.dma_start(dinv_sb[:], dinv_io[:])
            # two-region layout: tiles 0-48 on partition 0, 49-97 on
            # partition 64 (matmul lhsT base partition must be 0/32/64)
            rdinv_sb = constp.tile([65, 6272], dt.bfloat16)
            nc.sync.dma_start(rdinv_sb[0:1, :], rdinv_io[0:1, :])
            nc.sync.dma_start(rdinv_sb[64:65, :], rdinv_io[64:65, :])
            w1_sb = constp.tile([P, F0], dt.bfloat16)
            nc.sync.dma_start(w1_sb[:], w1_io[:])
            w2_sb = constp.tile([P, 64], dt.bfloat16)
            nc.sync.dma_start(w2_sb[:], w2_io[:])
            w3_sb = constp.tile([64, 32], dt.bfloat16)
            nc.sync.dma_start(w3_sb[:], w3_io[:])
            brow_sb = []
            for l in range(3):
                # bias row duplicated at partitions 0 and 64 to match the
                # two-region rdinv lhsT base partition
                bt = constp.tile([65, NCOLS[l]], dt.bfloat16, tag=f"brow{l}")
                nc.sync.dma_start(bt[0:1, :], brow_ios[l][0:1, :])
                nc.sync.dma_start(bt[64:65, :], brow_ios[l][64:65, :])
                brow_sb.append(bt)
            identb = constp.tile([P, P], dt.float32)
            nc.sync.dma_start(identb[:], identb_io[:])
            identf8 = constp.tile([P, P], SEL_DT)
            nc.sync.dma_start(identf8[:], identf8_io[:])
            identbb = constp.tile([P, P], dt.bfloat16)
            nc.sync.dma_start(identbb[:], identbb_io[:])

            hk = hkp.tile([P, NTILES * P], dt.bfloat16, tag="hk")

            bounces = [
                [
                    dramp.tile([SQR[q], FW], dt.bfloat16, tag=f"bnc{l}_{q}",
                               name=f"bnc{l}_{q}")
                    for q in range(4)
                ]
                for l in range(3)
            ]
            tbls = [
                [
                    dramp.tile([CH_ROWS[q], FW], dt.bfloat16, tag=f"tbl{l}_{q}",
                               name=f"tbl{l}_{q}",
                               addr_space="Shared" if TBL_SHARED else "Local")
                    for q in range(4)
                ]
                for l in range(3)
            ]

            def emit_ag(l, q):
                nc.gpsimd.collective_compute(
                    "AllGather", mybir.AluOpType.bypass,
                    replica_groups=[list(range(N_CORES))],
                    ins=[bounces[l][q].opt()], outs=[tbls[l][q].opt()],
                )

            # ---- dense0: hs1 = dinv * (x @ W1); grouped reads, quarter AGs ----
            nk = F0 // P
            DG = 8
            for t0 in range(0, NTILES, DG):
                nt = min(DG, NTILES - t0)
                ncol = min(NPC, (t0 + nt) * P) - t0 * P
                xts = []
                for k in range(nk):
                    xt = xTp.tile([P, DG * P], dt.bfloat16, tag="xt")
                    nc.sync.dma_start(
                        xt[:, :ncol], xT_io[k * P:(k + 1) * P, t0 * P:t0 * P + ncol]
                    )
                    xts.append(xt)
                for g in range(nt):
                    t = t0 + g
                    r = _rows(t)
                    c0 = t * P
                    q = _quarter_of_tile(t)
                    pd = psdp.tile([P, P], dt.float32, space="PSUM", tag="pd")
                    for k in range(nk):
                        nc.tensor.matmul(
                            out=pd[:r, :], lhsT=xts[k][:, g * P:g * P + r],
                            rhs=w1_sb[:, k * P:(k + 1) * P],
                            start=(k == 0), stop=(k == nk - 1),
                        )
                    nc.vector.tensor_scalar_mul(
                        hk[:r, c0:c0 + P], pd[:r, :], dinv_sb[:r, t:t + 1]
                    )
                    # bounce writes go on the Activation HWDGE queues so the
                    # AG triggers' completion-counter waits aren't polluted by
                    # sel/idx/gather traffic on the SP queues
                    nc.scalar.dma_start(
                        bounces[0][q][c0 - QSTART[q]:c0 - QSTART[q] + r, :],
                        hk[:r, c0:c0 + P],
                    )
                    if t == QT[q + 1] - 1:
                        emit_ag(0, q)

            # ---- fused aggregation + next dense, per layer ----
            # AG(0, q) triggers are interleaved into the layer-0 gather
            # emission (each before the first gather needing chunk q);
            # AG(l+1, q) triggers are emitted TRIG_LAG waves after quarter
            # q's aggregation waves are emitted, so the (in-order) gpsimd
            # queue doesn't stall waiting for its bounces.
            last_wave_of_q = {q: max(wi for wi, (qq, _) in enumerate(waves) if qq == q)
                              for q in range(4)}

            for l in range(3):
                ncols = NCOLS[l]
                fout = NCOLS[l + 1] if l < 2 else 0
                w_next = (w2_sb, w3_sb)[l] if l < 2 else None

                msg_tiles = {}   # (w, ch) -> tile
                sel_tiles = {}   # w -> tile

                def ensure_sel(w):
                    if w >= NW or w in sel_tiles:
                        return
                    # prefetch the wave's sel slice ahead of its aggregation
                    sb0 = int(wave_sel_start[w])
                    sbn = int(wave_sel_start[w + 1]) - sb0
                    selt = selp.tile([P, SEL_NB, P], SEL_DT, tag="sel")
                    sel_tiles[w] = selt
                    nc.sync.dma_start(
                        selt[:, :sbn, :], sel_io[:, sb0 * P:(sb0 + sbn) * P]
                    )

                def emit_gather(w, ch, l=l):
                    ensure_sel(w)
                    nb = int(call_nb[w, ch])
                    mt = msgp.tile([P, MSG_NB, FW], dt.bfloat16, tag="msg")
                    msg_tiles[(w, ch)] = mt
                    s0 = int(chunk_start[ch, w])
                    iw = idxp.tile([P, MSG_NB * 8], dt.int16, tag="idx")
                    nc.sync.dma_start(
                        iw[:, :nb * 8], idx_ios[ch][:, s0 // 16:(s0 + nb * P) // 16]
                    )
                    for b0 in range(0, nb, MAXB):
                        b1 = min(b0 + MAXB, nb)
                        Ssub = (b1 - b0) * P
                        nc.gpsimd.dma_gather(
                            out_ap=mt[:, b0:b1, :],
                            in_ap=tbls[l][ch][:, :],
                            idxs_ap=iw[:, b0 * 8:b1 * 8],
                            num_idxs=Ssub, num_idxs_reg=Ssub,
                            elem_size=FW, elem_step=FW,
                            single_packet=False,
                            queue_num=ch,
                        )

                def emit_agg(w, l=l, ncols=ncols, fout=fout, w_next=w_next):
                    q, wtiles = waves[w]
                    sb0 = int(wave_sel_start[w])
                    # pass 1: aggregation matmuls + relu for every tile, so
                    # the PE never stalls mid-wave on the scalar engine
                    outts = {}
                    for t in wtiles:
                        r = _rows(t)
                        c0 = t * P
                        pa = psap.tile([P, ncols], dt.float32, space="PSUM", tag="pa")
                        # self-loop rows from the local hs tile (PSUM init)
                        nc.tensor.matmul(
                            out=pa[:r, :], lhsT=identf8[:r, :r],
                            rhs=hk[:r, c0:c0 + ncols],
                            start=True, stop=False,
                        )
                        mms = []
                        for ch in range(NCH):
                            if not any_cnt[t, ch]:
                                continue
                            for j in range(int(blk0[t, ch]), int(blk1[t, ch])):
                                sp = int(sel_base[t, ch]) + j - int(blk0[t, ch])
                                mms.append((sp - sb0, ch, j))
                        # bias outer product: (1/dinv)[d] * b[f]; closes the
                        # accumulation group when a tile has no edge blocks
                        rp, rc = (0, t * P) if t < 49 else (64, (t - 49) * P)
                        nc.tensor.matmul(
                            out=pa[:r, :], lhsT=rdinv_sb[rp:rp + 1, rc:rc + r],
                            rhs=brow_sb[l][rp:rp + 1, :],
                            start=False, stop=(len(mms) == 0),
                        )
                        for i, (sj, ch, col) in enumerate(mms):
                            nc.tensor.matmul(
                                out=pa[:r, :], lhsT=sel_tiles[w][:, sj, :r],
                                rhs=msg_tiles[(w, ch)][:, col, :ncols],
                                start=False, stop=(i == len(mms) - 1),
                            )
                        if l < 2:
                            outt = outtp.tile([P, P], dt.bfloat16, tag="outt")
                            nc.scalar.activation(
                                outt[:r, :ncols], pa[:r, :],
                                mybir.ActivationFunctionType.Relu,
                                scale=dinv_sb[:r, t:t + 1],
                            )
                            outts[t] = outt
                        else:
                            o32 = outtp.tile([P, 32], dt.float32, tag="o32")
                            nc.scalar.activation(
                                o32[:r, :], pa[:r, :32],
                                mybir.ActivationFunctionType.Relu,
                                scale=dinv_sb[:r, t:t + 1],
                            )
                            nc.sync.dma_start(out_io[c0:c0 + r, :], o32[:r, :])
                    if l == 2:
                        return
                    # pass 2: transpose + next-layer dense per tile
                    for t in wtiles:
                        r = _rows(t)
                        c0 = t * P
                        tq = _quarter_of_tile(t)
                        outt = outts[t]
                        pst = pstp.tile([P, P], dt.bfloat16, space="PSUM", tag="pt")
                        nc.tensor.transpose(
                            out=pst[:ncols, :r], in_=outt[:r, :ncols],
                            identity=identbb[:r, :r],
                        )
                        att = attp.tile([P, P], dt.bfloat16, tag="att")
                        nc.scalar.copy(att[:ncols, :r], pst[:ncols, :r])
                        pd = psdp.tile([P, P], dt.float32, space="PSUM", tag="pd")
                        nc.tensor.matmul(
                            out=pd[:r, :fout], lhsT=att[:ncols, :r], rhs=w_next[:, :],
                            start=True, stop=True,
                        )
                        nc.vector.tensor_scalar_mul(
                            hk[:r, c0:c0 + fout], pd[:r, :fout], dinv_sb[:r, t:t + 1]
                        )
                        nc.scalar.dma_start(
                            bounces[l + 1][tq][c0 - QSTART[tq]:c0 - QSTART[tq] + r, :],
                            hk[:r, c0:c0 + P],
                        )

                # AG triggers for the NEXT layer, placed TRIG_LAG waves after
                # the quarter's last aggregation wave (or at the very end).
                trig_at = {}
                tail_qs = []
                if l < 2:
                    for q in range(4):
                        wpos = last_wave_of_q[q] + TRIG_LAG
                        if wpos < NW:
                            trig_at.setdefault(wpos, []).append(q)
                        else:
                            tail_qs.append(q)

                for w in range(NW):
                    if w == 0:
                        # chunk-major across the first two waves: each chunk's
                        # gathers grouped so a not-yet-ready chunk never
                        # blocks a ready one
                        for ch in range(3):
                            for wl in range(min(3, NW)):
                                emit_gather(wl, ch)
                    else:
                        if w + 1 < NW:
                            for ch in range(3):
                                emit_gather(w + 1, ch)
                    emit_gather(w, 3)
                    emit_agg(w)
                    ensure_sel(w + 2)
                    for q in trig_at.get(w, ()):
                        emit_ag(l + 1, q)
                for q in tail_qs:
                    emit_ag(l + 1, q)

    nc.compile()
    return nc


def _pack_inputs(x, dinv, W1, b1, W2, b2, W3, b3, idx16, sel):
    w1sb = np.zeros((P, F0), np.float32)
    for k in range(F0 // P):
        w1sb[:, k * P:(k + 1) * P] = W1[k * P:(k + 1) * P, :]
    identb = np.eye(P, dtype=np.float32)

    in_maps = []
    for c in range(N_CORES):
        lo = c * NPC
        xs = x[lo:lo + NPC].astype(np.float32)
        dv = dinv[lo:lo + NPC]
        dvt = np.ones((P, NTILES), np.float32)
        rdv = np.zeros((65, 6272), np.float32)
        for t in range(NTILES):
            r = _rows(t)
            dvt[:r, t] = dv[t * P:t * P + r]
            rp, rc = (0, t * P) if t < 49 else (64, (t - 49) * P)
            rdv[rp, rc:rc + r] = 1.0 / dv[t * P:t * P + r]
        in_maps.append({
            "xT": np.ascontiguousarray(xs.T).astype(BF),
            "dinvT": dvt,
            "rdinvR": rdv.astype(BF),
            "W1sb": w1sb.astype(BF),
            "W2sb": W2.astype(BF),
            "W3sb": W3.astype(BF),
            "b1row": np.tile(b1[None, :], (65, 1)).astype(BF),
            "b2row": np.tile(b2[None, :], (65, 1)).astype(BF),
            "b3row": np.tile(b3[None, :], (65, 1)).astype(BF),
            "identB": identb.astype(np.float32),
            "identBB": identb.astype(BF),
            "identF8": identb.astype(SEL_NP),
            **{f"idx{ch}": idx16[ch][c] for ch in range(NCH)},
            "sel": sel[c].reshape(P, -1),
        })
    return in_maps


_TRACE = [False]          # set by test harness to request a profiled run
_LAST_RESULT = [None]     # BassKernelResults of the last run (for profiling)


def kernel(x, edge_index, batch, W1, b1, W2, b2, W3, b3, Wfc, bfc):
    x = np.asarray(x)
    edge_index = np.asarray(edge_index)
    batch = np.asarray(batch)
    W1, b1 = np.asarray(W1), np.asarray(b1)
    W2, b2 = np.asarray(W2), np.asarray(b2)
    W3, b3 = np.asarray(W3), np.asarray(b3)
    Wfc, bfc = np.asarray(Wfc), np.asarray(bfc)

    dinv, idx16, sel, meta = _host_prep(edge_index.astype(np.int64))
    nc = _build_program(meta)
    in_maps = _pack_inputs(x, dinv, W1, b1, W2, b2, W3, b3, idx16, sel)
    res = run_bass_kernel_spmd(
        nc, in_maps, core_ids=list(range(N_CORES)), trace=_TRACE[0]
    )
    _LAST_RESULT[0] = res

    h3 = np.concatenate([res.results[c]["out3"] for c in range(N_CORES)], axis=0)

    # host epilogue: segment max pool + FC + log_softmax (float64 for stability)
    pooled = np.full((N_GRAPHS, 32), -np.inf, np.float64)
    bnd = np.searchsorted(batch, np.arange(N_GRAPHS + 1))
    for g in range(N_GRAPHS):
        if bnd[g + 1] > bnd[g]:
            pooled[g] = h3[bnd[g]:bnd[g + 1]].max(axis=0)
    logits = pooled @ Wfc.astype(np.float64) + bfc.astype(np.float64)
    m = logits.max(axis=1, keepdims=True)
    lse = m + np.log(np.exp(logits - m).sum(axis=1, keepdims=True))
    return (logits - lse).astype(np.float32)


# revision 42
# speedup vs baseline: 1.0132x; 1.0132x over previous
"""GCN (3x GCNConv + global max pool + FC + log_softmax) on 8 Trainium2 NeuronCores.

Strategy (v3, continuous packing + chunk-major scheduling):
  - 1D partition of nodes: core c owns rows [12500c, 12500(c+1)).
  - Table for layer l holds hs_l = dinv * (a_l @ W_l) rows (bf16, 128-wide,
    256B gather elems), split into 4 chunks (source quarters), each the
    AllGather (quarter-interleaved across cores) of one quarter of every
    core's rows.
  - Aggregation per 128-dst tile via gpsimd dma_gather + PE one-hot sum.
    v3: slots are packed CONTINUOUSLY within each (wave, chunk) gather
    call (tiles back-to-back, one 128-roundup per call) instead of
    rounding every (tile, chunk) group to 128-slot blocks. Tile segments
    may straddle msg blocks; each (tile, overlapped-block) pair gets its
    own host-built one-hot sel matrix (per-core data), so the static
    instruction stream is core-independent while per-core edge counts
    differ. Cuts gather descriptors (the gpsimd bottleneck) ~14%.
  - v3 scheduling: AllGather triggers (which run on gpsimd and WAIT for
    their source bounces) are interleaved with gather emission so the
    in-order gpsimd queue never head-of-line blocks: at layer starts the
    ch<3 gathers of the first waves are emitted chunk-major before the
    first ch3 gather, and each AG trigger is emitted only after the wave
    whose PE work already covers its source quarter.
  - Self-loops never gathered: identity-matmul from the SBUF-resident
    local hs tile; bias via a K=1 outer-product matmul.
  - Post per tile fused: scalar relu(dinv * psum) -> bf16, PE transpose,
    scalar copy, next layer's dense matmul + dinv scale immediately.
  - Pooling (segment max over graphs), tiny FC and log_softmax on host.
"""

import os
import sys

sys.path.insert(0, "/opt/trn_rl_repo")

import numpy as np
import ml_dtypes

import concourse.bass as bass
import concourse.bacc as bacc
import concourse.tile as tile
from concourse import mybir
from concourse.bass_utils import run_bass_kernel_spmd

P = 128
N_NODES = 100000
N_EDGES = 1600000
N_GRAPHS = 64
N_CORES = 8
NPC = N_NODES // N_CORES                     # 12500
NTILES = (NPC + P - 1) // P                  # 98 (last tile 84 rows)
LAST_ROWS = NPC - (NTILES - 1) * P           # 84
F0 = 512
FW = 128                                     # table row width (256B gather elem)
NCH = 4                                      # chunks = src quarters of each core
QT = [0, 25, 50, 75, 98]                     # quarter tile boundaries
QSTART = [0, 3200, 6400, 9600]               # quarter row starts
QROWS = [3200, 3200, 3200, 2900]             # rows per quarter per core
SQR = [3200, 3200, 3200, 2944]               # 128-aligned per-core chunk stride
CH_ROWS = [N_CORES * r for r in SQR]         # table rows per chunk
NCOLS = [128, 64, 32]                        # real table width per layer
WT = int(os.environ.get("GCN_WT", "7"))      # target tiles per wave
MAXB = int(os.environ.get("GCN_MAXB", "34"))  # max blocks per dma_gather call
NQUEUES = 4
DMA_SCRATCH = int(os.environ.get("GCN_SCRATCH", "30720"))
SEL_F8 = os.environ.get("GCN_SELDT", "f8") == "f8"
TBL_SHARED = os.environ.get("GCN_SHARED", "1") == "1"
MSG_BUFS = int(os.environ.get("GCN_MSGB", "13"))
SEL_BUFS = int(os.environ.get("GCN_SELB", "2"))
TRIG_LAG = int(os.environ.get("GCN_TRIGLAG", "2"))  # waves of PE lag before AG trigger
dt = mybir.dt
BF = ml_dtypes.bfloat16
F8 = ml_dtypes.float8_e4m3
SEL_DT = dt.float8e4 if SEL_F8 else dt.bfloat16
SEL_NP = F8 if SEL_F8 else BF


def _rows(t):
    return LAST_ROWS if t == NTILES - 1 else P


def _quarter_of_tile(t):
    for q in range(4):
        if t < QT[q + 1]:
            return q
    raise AssertionError


def _make_waves():
    waves = []
    for q in range(4):
        tiles = list(range(QT[q], QT[q + 1]))
        nw = (len(tiles) + WT - 1) // WT
        for part in np.array_split(np.array(tiles), nw):
            waves.append((q, [int(t) for t in part]))
    return waves


def _host_prep(edge_index):
    """Shared (cross-core) aggregation schedule + per-core index/sel data."""
    src = edge_index[0].astype(np.int64)
    dst = edge_index[1].astype(np.int64)
    deg = np.bincount(dst, minlength=N_NODES).astype(np.float64) + 1.0
    dinv = (1.0 / np.sqrt(deg)).astype(np.float32)

    waves = _make_waves()
    NW = len(waves)
    wave_of_tile = np.zeros(NTILES, np.int64)
    for wi, (q, ts) in enumerate(waves):
        for t in ts:
            wave_of_tile[t] = wi

    # chunk position of a src node: quarter-interleaved table layout
    s_core = src // NPC
    s_loc = src % NPC
    s_tile = s_loc // P
    s_q = np.digitize(s_tile, QT[1:4])              # 0..3
    qrows = np.array(SQR)[s_q]
    qstart = np.array(QSTART)[s_q]
    s_pos = s_core * qrows + (s_loc - qstart)       # position within chunk s_q

    core_of = dst // NPC

    # per-core counts per (wave, chunk, tile)
    cnt3 = np.zeros((N_CORES, NW, NCH, NTILES), np.int64)
    per_core_raw = []
    for c in range(N_CORES):
        m = core_of == c
        pos, chv = s_pos[m], s_q[m]
        dl = dst[m] - c * NPC
        t = dl // P
        w = wave_of_tile[t]
        key = (w * NCH + chv) * NTILES + t
        o = np.argsort(key, kind="stable")
        pos, chv, t, dl, w, key = pos[o], chv[o], t[o], dl[o], w[o], key[o]
        cnt3[c] = np.bincount(key, minlength=NW * NCH * NTILES).reshape(
            NW, NCH, NTILES
        )
        per_core_raw.append((pos, chv, t, dl - t * P, w, key))

    # within-call offsets of each tile segment (per core), call sizes
    off_tc = np.zeros((N_CORES, NTILES, NCH), np.int64)
    cnt_tc = np.zeros((N_CORES, NTILES, NCH), np.int64)
    cnt_call = np.zeros((N_CORES, NW, NCH), np.int64)
    for c in range(N_CORES):
        for wi, (q, ts) in enumerate(waves):
            for ch in range(NCH):
                acc = 0
                for t in ts:
                    off_tc[c, t, ch] = acc
                    cnt_tc[c, t, ch] = cnt3[c, wi, ch, t]
                    acc += cnt3[c, wi, ch, t]
                cnt_call[c, wi, ch] = acc
    call_nb = np.maximum((cnt_call.max(axis=0) + P - 1) // P, 1)  # [NW, NCH]

    # static per-(tile, chunk) block span (union over cores)
    blk0 = np.zeros((NTILES, NCH), np.int64)
    blk1 = np.zeros((NTILES, NCH), np.int64)
    any_cnt = cnt_tc.max(axis=0) > 0                # [NTILES, NCH]
    for t in range(NTILES):
        for ch in range(NCH):
            if not any_cnt[t, ch]:
                continue
            s = off_tc[:, t, ch]
            e = s + cnt_tc[:, t, ch]
            act = cnt_tc[:, t, ch] > 0
            blk0[t, ch] = (s[act] // P).min()
            blk1[t, ch] = ((e[act] + P - 1) // P).max()

    # sel order: (wave, tile, chunk, block)
    sel_base = np.zeros((NTILES, NCH), np.int64)
    wave_sel_start = np.zeros(NW + 1, np.int64)
    g = 0
    for wi, (q, ts) in enumerate(waves):
        wave_sel_start[wi] = g
        for t in ts:
            for ch in range(NCH):
                if not any_cnt[t, ch]:
                    continue
                sel_base[t, ch] = g
                g += int(blk1[t, ch] - blk0[t, ch])
    wave_sel_start[NW] = g
    NB_SEL = g

    # idx slot order per chunk: (wave, block)
    chunk_start = np.zeros((NCH, NW), np.int64)
    S_ch = [0] * NCH
    for ch in range(NCH):
        acc = 0
        for wi in range(NW):
            chunk_start[ch, wi] = acc
            acc += int(call_nb[wi, ch]) * P
        S_ch[ch] = acc

    idx16 = [np.zeros((N_CORES, P, S_ch[ch] // 16), np.int16) for ch in range(NCH)]
    sel = np.zeros((N_CORES, P, NB_SEL, P), SEL_NP)

    for c in range(N_CORES):
        pos, chv, t, dd, w, key = per_core_raw[c]
        flat = np.bincount(key, minlength=NW * NCH * NTILES)
        starts = np.cumsum(flat) - flat
        i = np.arange(len(pos)) - starts[key]       # seq within (w, ch, t)
        slot_in_call = off_tc[c, t, chv] + i
        j = slot_in_call // P
        lane = slot_in_call % P
        selpos = sel_base[t, chv] + (j - blk0[t, chv])
        sel[c][lane, selpos, dd] = 1.0
        chunk_slot = chunk_start[chv, w] + slot_in_call
        for ch in range(NCH):
            mm = chv == ch
            slots_arr = np.zeros(S_ch[ch], np.int16)
            slots_arr[chunk_slot[mm]] = pos[mm].astype(np.int16)
            idx16[ch][c] = np.tile(slots_arr.reshape(-1, 16).T, (8, 1))

    meta = {
        "waves": waves,
        "call_nb": call_nb,
        "blk0": blk0,
        "blk1": blk1,
        "any_cnt": any_cnt,
        "sel_base": sel_base,
        "wave_sel_start": wave_sel_start,
        "NB_SEL": NB_SEL,
        "chunk_start": chunk_start,
        "S_ch": S_ch,
    }
    return dinv, idx16, sel, meta


def _build_program(meta):
    waves = meta["waves"]
    call_nb = meta["call_nb"]
    blk0 = meta["blk0"]
    blk1 = meta["blk1"]
    any_cnt = meta["any_cnt"]
    sel_base = meta["sel_base"]
    wave_sel_start = meta["wave_sel_start"]
    NB_SEL = meta["NB_SEL"]
    chunk_start = meta["chunk_start"]
    S_ch = meta["S_ch"]
    NW = len(waves)

    MSG_NB = int(call_nb.max())
    SEL_NB = int(np.diff(wave_sel_start).max())

    nc = bacc.Bacc(
        "TRN2", target_bir_lowering=False, debug=False, num_devices=N_CORES,
        num_swdge_queues=NQUEUES, dynamic_dma_scratch_size=DMA_SCRATCH,
    )

    xT_io = nc.dram_tensor("xT", [F0, NPC], dt.bfloat16, kind="ExternalInput").ap()
    dinv_io = nc.dram_tensor("dinvT", [P, NTILES], dt.float32, kind="ExternalInput").ap()
    rdinv_io = nc.dram_tensor("rdinvR", [65, 6272], dt.bfloat16, kind="ExternalInput").ap()
    w1_io = nc.dram_tensor("W1sb", [P, F0], dt.bfloat16, kind="ExternalInput").ap()
    w2_io = nc.dram_tensor("W2sb", [P, 64], dt.bfloat16, kind="ExternalInput").ap()
    w3_io = nc.dram_tensor("W3sb", [64, 32], dt.bfloat16, kind="ExternalInput").ap()
    brow_ios = [
        nc.dram_tensor(f"b{l+1}row", [65, NCOLS[l]], dt.bfloat16, kind="ExternalInput").ap()
        for l in range(3)
    ]
    identb_io = nc.dram_tensor("identB", [P, P], dt.float32, kind="ExternalInput").ap()
    identf8_io = nc.dram_tensor("identF8", [P, P], SEL_DT, kind="ExternalInput").ap()
    identbb_io = nc.dram_tensor("identBB", [P, P], dt.bfloat16, kind="ExternalInput").ap()
    idx_ios = [
        nc.dram_tensor(f"idx{ch}", [P, S_ch[ch] // 16], dt.int16, kind="ExternalInput").ap()
        for ch in range(NCH)
    ]
    sel_io = nc.dram_tensor("sel", [P, NB_SEL * P], SEL_DT, kind="ExternalInput").ap()
    out_io = nc.dram_tensor("out3", [NPC, 32], dt.float32, kind="ExternalOutput").ap()

    with tile.TileContext(nc) as tc:
        with (
            tc.tile_pool(name="const", bufs=1) as constp,
            tc.tile_pool(name="hskeep", bufs=1) as hkp,
            tc.tile_pool(name="xT", bufs=4) as xTp,
            tc.tile_pool(name="idxw", bufs=7) as idxp,
            tc.tile_pool(name="msgs", bufs=MSG_BUFS) as msgp,
            tc.tile_pool(name="sel", bufs=SEL_BUFS) as selp,
            tc.tile_pool(name="outt", bufs=7) as outtp,
            tc.tile_pool(name="att", bufs=3) as attp,
            tc.tile_pool(name="psum_a", bufs=4, space="PSUM") as psap,
            tc.tile_pool(name="psum_t", bufs=2, space="PSUM") as pstp,
            tc.tile_pool(name="psum_d", bufs=2, space="PSUM") as psdp,
            tc.tile_pool(name="dram", bufs=1, space="DRAM") as dramp,
        ):
            # ---- constants ----
            dinv_sb = constp.tile([P, NTILES], dt.float32)
            nc.sync.dma_start(dinv_sb[:], dinv_io[:])
            # two-region layout: tiles 0-48 on partition 0, 49-97 on
            # partition 64 (matmul lhsT base partition must be 0/32/64)
            rdinv_sb = constp.tile([65, 6272], dt.bfloat16)
            nc.sync.dma_start(rdinv_sb[0:1, :], rdinv_io[0:1, :])
            nc.sync.dma_start(rdinv_sb[64:65, :], rdinv_io[64:65, :])
            w1_sb = constp.tile([P, F0], dt.bfloat16)
            nc.sync.dma_start(w1_sb[:], w1_io[:])
            w2_sb = constp.tile([P, 64], dt.bfloat16)
            nc.sync.dma_start(w2_sb[:], w2_io[:])
            w3_sb = constp.tile([64, 32], dt.bfloat16)
            nc.sync.dma_start(w3_sb[:], w3_io[:])
            brow_sb = []
            for l in range(3):
                # bias row duplicated at partitions 0 and 64 to match the
                # two-region rdinv lhsT base partition
                bt = constp.tile([65, NCOLS[l]], dt.bfloat16, tag=f"brow{l}")
                nc.sync.dma_start(bt[0:1, :], brow_ios[l][0:1, :])
                nc.sync.dma_start(bt[64:65, :], brow_ios[l][64:65, :])
                brow_sb.append(bt)
            identb = constp.tile([P, P], dt.float32)
            nc.sync.dma_start(identb[:], identb_io[:])
            identf8 = constp.tile([P, P], SEL_DT)
            nc.sync.dma_start(identf8[:], identf8_io[:])
            identbb = constp.tile([P, P], dt.bfloat16)
            nc.sync.dma_start(identbb[:], identbb_io[:])

            hk = hkp.tile([P, NTILES * P], dt.bfloat16, tag="hk")

            bounces = [
                [
                    dramp.tile([SQR[q], FW], dt.bfloat16, tag=f"bnc{l}_{q}",
                               name=f"bnc{l}_{q}")
                    for q in range(4)
                ]
                for l in range(3)
            ]
            tbls = [
                [
                    dramp.tile([CH_ROWS[q], FW], dt.bfloat16, tag=f"tbl{l}_{q}",
                               name=f"tbl{l}_{q}",
                               addr_space="Shared" if TBL_SHARED else "Local")
                    for q in range(4)
                ]
                for l in range(3)
            ]

            def emit_ag(l, q):
                nc.gpsimd.collective_compute(
                    "AllGather", mybir.AluOpType.bypass,
                    replica_groups=[list(range(N_CORES))],
                    ins=[bounces[l][q].opt()], outs=[tbls[l][q].opt()],
                )

            # ---- dense0: hs1 = dinv * (x @ W1); grouped reads, quarter AGs ----
            nk = F0 // P
            DG = 8
            for t0 in range(0, NTILES, DG):
                nt = min(DG, NTILES - t0)
                ncol = min(NPC, (t0 + nt) * P) - t0 * P
                xts = []
                for k in range(nk):
                    xt = xTp.tile([P, DG * P], dt.bfloat16, tag="xt")
                    nc.sync.dma_start(
                        xt[:, :ncol], xT_io[k * P:(k + 1) * P, t0 * P:t0 * P + ncol]
                    )
                    xts.append(xt)
                for g in range(nt):
                    t = t0 + g
                    r = _rows(t)
                    c0 = t * P
                    q = _quarter_of_tile(t)
                    pd = psdp.tile([P, P], dt.float32, space="PSUM", tag="pd")
                    for k in range(nk):
                        nc.tensor.matmul(
                            out=pd[:r, :], lhsT=xts[k][:, g * P:g * P + r],
                            rhs=w1_sb[:, k * P:(k + 1) * P],
                            start=(k == 0), stop=(k == nk - 1),
                        )
                    nc.vector.tensor_scalar_mul(
                        hk[:r, c0:c0 + P], pd[:r, :], dinv_sb[:r, t:t + 1]
                    )
                    # bounce writes go on the Activation HWDGE queues so the
                    # AG triggers' completion-counter waits aren't polluted by
                    # sel/idx/gather traffic on the SP queues
                    nc.scalar.dma_start(
                        bounces[0][q][c0 - QSTART[q]:c0 - QSTART[q] + r, :],
                        hk[:r, c0:c0 + P],
                    )
                    if t == QT[q + 1] - 1:
                        emit_ag(0, q)

            # ---- fused aggregation + next dense, per layer ----
            # AG(0, q) triggers are interleaved into the layer-0 gather
            # emission (each before the first gather needing chunk q);
            # AG(l+1, q) triggers are emitted TRIG_LAG waves after quarter
            # q's aggregation waves are emitted, so the (in-order) gpsimd
            # queue doesn't stall waiting for its bounces.
            last_wave_of_q = {q: max(wi for wi, (qq, _) in enumerate(waves) if qq == q)
                              for q in range(4)}

            for l in range(3):
                ncols = NCOLS[l]
                fout = NCOLS[l + 1] if l < 2 else 0
                w_next = (w2_sb, w3_sb)[l] if l < 2 else None

                msg_tiles = {}   # (w, ch) -> tile
                sel_tiles = {}   # w -> tile

                def ensure_sel(w):
                    if w >= NW or w in sel_tiles:
                        return
                    # prefetch the wave's sel slice ahead of its aggregation
                    sb0 = int(wave_sel_start[w])
                    sbn = int(wave_sel_start[w + 1]) - sb0
                    selt = selp.tile([P, SEL_NB, P], SEL_DT, tag="sel")
                    sel_tiles[w] = selt
                    nc.sync.dma_start(
                        selt[:, :sbn, :], sel_io[:, sb0 * P:(sb0 + sbn) * P]
                    )

                def emit_gather(w, ch, l=l):
                    ensure_sel(w)
                    nb = int(call_nb[w, ch])
                    mt = msgp.tile([P, MSG_NB, FW], dt.bfloat16, tag="msg")
                    msg_tiles[(w, ch)] = mt
                    s0 = int(chunk_start[ch, w])
                    iw = idxp.tile([P, MSG_NB * 8], dt.int16, tag="idx")
                    nc.sync.dma_start(
                        iw[:, :nb * 8], idx_ios[ch][:, s0 // 16:(s0 + nb * P) // 16]
                    )
                    for b0 in range(0, nb, MAXB):
                        b1 = min(b0 + MAXB, nb)
                        Ssub = (b1 - b0) * P
                        nc.gpsimd.dma_gather(
                            out_ap=mt[:, b0:b1, :],
                            in_ap=tbls[l][ch][:, :],
                            idxs_ap=iw[:, b0 * 8:b1 * 8],
                            num_idxs=Ssub, num_idxs_reg=Ssub,
                            elem_size=FW, elem_step=FW,
                            single_packet=False,
                            queue_num=ch,
                        )

                def emit_agg(w, l=l, ncols=ncols, fout=fout, w_next=w_next):
                    q, wtiles = waves[w]
                    sb0 = int(wave_sel_start[w])
                    # pass 1: aggregation matmuls + relu for every tile, so
                    # the PE never stalls mid-wave on the scalar engine
                    outts = {}
                    for t in wtiles:
                        r = _rows(t)
                        c0 = t * P
                        pa = psap.tile([P, ncols], dt.float32, space="PSUM", tag="pa")
                        # self-loop rows from the local hs tile (PSUM init)
                        nc.tensor.matmul(
                            out=pa[:r, :], lhsT=identf8[:r, :r],
                            rhs=hk[:r, c0:c0 + ncols],
                            start=True, stop=False,
                        )
                        mms = []
                        for ch in range(NCH):
                            if not any_cnt[t, ch]:
                                continue
                            for j in range(int(blk0[t, ch]), int(blk1[t, ch])):
                                sp = int(sel_base[t, ch]) + j - int(blk0[t, ch])
                                mms.append((sp - sb0, ch, j))
                        # bias outer product: (1/dinv)[d] * b[f]; closes the
                        # accumulation group when a tile has no edge blocks
                        rp, rc = (0, t * P) if t < 49 else (64, (t - 49) * P)
                        nc.tensor.matmul(
                            out=pa[:r, :], lhsT=rdinv_sb[rp:rp + 1, rc:rc + r],
                            rhs=brow_sb[l][rp:rp + 1, :],
                            start=False, stop=(len(mms) == 0),
                        )
                        for i, (sj, ch, col) in enumerate(mms):
                            nc.tensor.matmul(
                                out=pa[:r, :], lhsT=sel_tiles[w][:, sj, :r],
                                rhs=msg_tiles[(w, ch)][:, col, :ncols],
                                start=False, stop=(i == len(mms) - 1),
                            )
                        if l < 2:
                            outt = outtp.tile([P, P], dt.bfloat16, tag="outt")
                            nc.scalar.activation(
                                outt[:r, :ncols], pa[:r, :],
                                mybir.ActivationFunctionType.Relu,
                                scale=dinv_sb[:r, t:t + 1],
                            )
                            outts[t] = outt
                        else:
                            o32 = outtp.tile([P, 32], dt.float32, tag="o32")
                            nc.scalar.activation(
                                o32[:r, :], pa[:r, :32],
                                mybir.ActivationFunctionType.Relu,
                                scale=dinv_sb[:r, t:t + 1],
                            )
                            nc.sync.dma_start(out_io[c0:c0 + r, :], o32[:r, :])
                    if l == 2:
                        return
                    # pass 2: transpose + next-layer dense per tile
                    for t in wtiles:
                        r = _rows(t)
                        c0 = t * P
                        tq = _quarter_of_tile(t)
                        outt = outts[t]
                        pst = pstp.tile([P, P], dt.bfloat16, space="PSUM", tag="pt")
                        nc.tensor.transpose(
                            out=pst[:ncols, :r], in_=outt[:r, :ncols],
                            identity=identbb[:r, :r],
                        )
                        att = attp.tile([P, P], dt.bfloat16, tag="att")
                        nc.scalar.copy(att[:ncols, :r], pst[:ncols, :r])
                        pd = psdp.tile([P, P], dt.float32, space="PSUM", tag="pd")
                        nc.tensor.matmul(
                            out=pd[:r, :fout], lhsT=att[:ncols, :r], rhs=w_next[:, :],
                            start=True, stop=True,
                        )
                        nc.vector.tensor_scalar_mul(
                            hk[:r, c0:c0 + fout], pd[:r, :fout], dinv_sb[:r, t:t + 1]
                        )
                        nc.scalar.dma_start(
                            bounces[l + 1][tq][c0 - QSTART[tq]:c0 - QSTART[tq] + r, :],
                            hk[:r, c0:c0 + P],
                        )

                # AG triggers for the NEXT layer, placed TRIG_LAG waves after
                # the quarter's last aggregation wave (or at the very end).
                trig_at = {}
                tail_qs = []
                if l < 2:
                    for q in range(4):
                        wpos = last_wave_of_q[q] + TRIG_LAG
                        if wpos < NW:
                            trig_at.setdefault(wpos, []).append(q)
                        else:
                            tail_qs.append(q)

                for w in range(NW):
                    if w == 0:
                        # chunk-major across the first two waves: each chunk's
                        # gathers grouped so a not-yet-ready chunk never
                        # blocks a ready one
                        for ch in range(3):
                            for wl in range(min(3, NW)):
                                emit_gather(wl, ch)
                    else:
                        if w + 1 < NW:
                            for ch in range(3):
                                emit_gather(w + 1, ch)
                    emit_gather(w, 3)
                    emit_agg(w)
                    ensure_sel(w + 2)
                    for q in trig_at.get(w, ()):
                        emit_ag(l + 1, q)
                for q in tail_qs:
                    emit_ag(l + 1, q)

    nc.compile()
    return nc


def _pack_inputs(x, dinv, W1, b1, W2, b2, W3, b3, idx16, sel):
    w1sb = np.zeros((P, F0), np.float32)
    for k in range(F0 // P):
        w1sb[:, k * P:(k + 1) * P] = W1[k * P:(k + 1) * P, :]
    identb = np.eye(P, dtype=np.float32)

    in_maps = []
    for c in range(N_CORES):
        lo = c * NPC
        xs = x[lo:lo + NPC].astype(np.float32)
        dv = dinv[lo:lo + NPC]
        dvt = np.ones((P, NTILES), np.float32)
        rdv = np.zeros((65, 6272), np.float32)
        for t in range(NTILES):
            r = _rows(t)
            dvt[:r, t] = dv[t * P:t * P + r]
            rp, rc = (0, t * P) if t < 49 else (64, (t - 49) * P)
            rdv[rp, rc:rc + r] = 1.0 / dv[t * P:t * P + r]
        in_maps.append({
            "xT": np.ascontiguousarray(xs.T).astype(BF),
            "dinvT": dvt,
            "rdinvR": rdv.astype(BF),
            "W1sb": w1sb.astype(BF),
            "W2sb": W2.astype(BF),
            "W3sb": W3.astype(BF),
            "b1row": np.tile(b1[None, :], (65, 1)).astype(BF),
            "b2row": np.tile(b2[None, :], (65, 1)).astype(BF),
            "b3row": np.tile(b3[None, :], (65, 1)).astype(BF),
            "identB": identb.astype(np.float32),
            "identBB": identb.astype(BF),
            "identF8": identb.astype(SEL_NP),
            **{f"idx{ch}": idx16[ch][c] for ch in range(NCH)},
            "sel": sel[c].reshape(P, -1),
        })
    return in_maps


_TRACE = [False]          # set by test harness to request a profiled run
_LAST_RESULT = [None]     # BassKernelResults of the last run (for profiling)


def kernel(x, edge_index, batch, W1, b1, W2, b2, W3, b3, Wfc, bfc):
    x = np.asarray(x)
    edge_index = np.asarray(edge_index)
    batch = np.asarray(batch)
    W1, b1 = np.asarray(W1), np.asarray(b1)
    W2, b2 = np.asarray(W2), np.asarray(b2)
    W3, b3 = np.asarray(W3), np.asarray(b3)
    Wfc, bfc = np.asarray(Wfc), np.asarray(bfc)

    dinv, idx16, sel, meta = _host_prep(edge_index.astype(np.int64))
    nc = _build_program(meta)
    in_maps = _pack_inputs(x, dinv, W1, b1, W2, b2, W3, b3, idx16, sel)
    res = run_bass_kernel_spmd(
        nc, in_maps, core_ids=list(range(N_CORES)), trace=_TRACE[0]
    )
    _LAST_RESULT[0] = res

    h3 = np.concatenate([res.results[c]["out3"] for c in range(N_CORES)], axis=0)

    # host epilogue: segment max pool + FC + log_softmax (float64 for stability)
    pooled = np.full((N_GRAPHS, 32), -np.inf, np.float64)
    bnd = np.searchsorted(batch, np.arange(N_GRAPHS + 1))
    for g in range(N_GRAPHS):
        if bnd[g + 1] > bnd[g]:
            pooled[g] = h3[bnd[g]:bnd[g + 1]].max(axis=0)
    logits = pooled @ Wfc.astype(np.float64) + bfc.astype(np.float64)
    m = logits.max(axis=1, keepdims=True)
    lse = m + np.log(np.exp(logits - m).sum(axis=1, keepdims=True))
    return (logits - lse).astype(np.float32)


# revision 46
# speedup vs baseline: 1.1075x; 1.0931x over previous
"""GCN (3x GCNConv + global max pool + FC + log_softmax) on 8 Trainium2 NeuronCores.

Strategy (v3, continuous packing + chunk-major scheduling):
  - 1D partition of nodes: core c owns rows [12500c, 12500(c+1)).
  - Table for layer l holds hs_l = dinv * (a_l @ W_l) rows (bf16, 128-wide,
    256B gather elems), split into 4 chunks (source quarters), each the
    AllGather (quarter-interleaved across cores) of one quarter of every
    core's rows.
  - Aggregation per 128-dst tile via gpsimd dma_gather + PE one-hot sum.
    v3: slots are packed CONTINUOUSLY within each (wave, chunk) gather
    call (tiles back-to-back, one 128-roundup per call) instead of
    rounding every (tile, chunk) group to 128-slot blocks. Tile segments
    may straddle msg blocks; each (tile, overlapped-block) pair gets its
    own host-built one-hot sel matrix (per-core data), so the static
    instruction stream is core-independent while per-core edge counts
    differ. Cuts gather descriptors (the gpsimd bottleneck) ~14%.
  - v3 scheduling: AllGather triggers (which run on gpsimd and WAIT for
    their source bounces) are interleaved with gather emission so the
    in-order gpsimd queue never head-of-line blocks: at layer starts the
    ch<3 gathers of the first waves are emitted chunk-major before the
    first ch3 gather, and each AG trigger is emitted only after the wave
    whose PE work already covers its source quarter.
  - Self-loops never gathered: identity-matmul from the SBUF-resident
    local hs tile; bias via a K=1 outer-product matmul.
  - Post per tile fused: scalar relu(dinv * psum) -> bf16, PE transpose,
    scalar copy, next layer's dense matmul + dinv scale immediately.
  - Pooling (segment max over graphs), tiny FC and log_softmax on host.
"""

import os
import sys

sys.path.insert(0, "/opt/trn_rl_repo")

import numpy as np
import ml_dtypes

import concourse.bass as bass
import concourse.bacc as bacc
import concourse.tile as tile
from concourse import mybir
from concourse.bass_utils import run_bass_kernel_spmd

P = 128
N_NODES = 100000
N_EDGES = 1600000
N_GRAPHS = 64
N_CORES = 8
NPC = N_NODES // N_CORES                     # 12500
NTILES = (NPC + P - 1) // P                  # 98 (last tile 84 rows)
LAST_ROWS = NPC - (NTILES - 1) * P           # 84
F0 = 512
FW = 128                                     # table row width (256B gather elem)
NCH = 4                                      # chunks = src quarters of each core
QT = [0, 25, 50, 75, 98]                     # quarter tile boundaries
QSTART = [0, 3200, 6400, 9600]               # quarter row starts
QROWS = [3200, 3200, 3200, 2900]             # rows per quarter per core
SQR = [3200, 3200, 3200, 2944]               # 128-aligned per-core chunk stride
CH_ROWS = [N_CORES * r for r in SQR]         # table rows per chunk
NCOLS = [128, 64, 32]                        # real table width per layer
WT = int(os.environ.get("GCN_WT", "7"))      # target tiles per wave
MAXB = int(os.environ.get("GCN_MAXB", "34"))  # max blocks per dma_gather call
NQUEUES = 4
DMA_SCRATCH = int(os.environ.get("GCN_SCRATCH", "30720"))
SEL_F8 = os.environ.get("GCN_SELDT", "f8") == "f8"
TBL_SHARED = os.environ.get("GCN_SHARED", "1") == "1"
MSG_BUFS = int(os.environ.get("GCN_MSGB", "13"))
SEL_BUFS = int(os.environ.get("GCN_SELB", "2"))
TRIG_LAG = int(os.environ.get("GCN_TRIGLAG", "2"))  # waves of PE lag before AG trigger
dt = mybir.dt
BF = ml_dtypes.bfloat16
F8 = ml_dtypes.float8_e4m3
SEL_DT = dt.float8e4 if SEL_F8 else dt.bfloat16
SEL_NP = F8 if SEL_F8 else BF


def _rows(t):
    return LAST_ROWS if t == NTILES - 1 else P


def _quarter_of_tile(t):
    for q in range(4):
        if t < QT[q + 1]:
            return q
    raise AssertionError


def _make_waves():
    waves = []
    for q in range(4):
        tiles = list(range(QT[q], QT[q + 1]))
        nw = (len(tiles) + WT - 1) // WT
        for part in np.array_split(np.array(tiles), nw):
            waves.append((q, [int(t) for t in part]))
    return waves


def _host_prep(edge_index):
    """Shared (cross-core) aggregation schedule + per-core index/sel data."""
    src = edge_index[0].astype(np.int64)
    dst = edge_index[1].astype(np.int64)
    deg = np.bincount(dst, minlength=N_NODES).astype(np.float64) + 1.0
    dinv = (1.0 / np.sqrt(deg)).astype(np.float32)

    waves = _make_waves()
    NW = len(waves)
    wave_of_tile = np.zeros(NTILES, np.int64)
    for wi, (q, ts) in enumerate(waves):
        for t in ts:
            wave_of_tile[t] = wi

    # chunk position of a src node: quarter-interleaved table layout
    s_core = src // NPC
    s_loc = src % NPC
    s_tile = s_loc // P
    s_q = np.digitize(s_tile, QT[1:4])              # 0..3
    qrows = np.array(SQR)[s_q]
    qstart = np.array(QSTART)[s_q]
    s_pos = s_core * qrows + (s_loc - qstart)       # position within chunk s_q

    core_of = dst // NPC

    # per-core counts per (wave, chunk, tile)
    cnt3 = np.zeros((N_CORES, NW, NCH, NTILES), np.int64)
    per_core_raw = []
    for c in range(N_CORES):
        m = core_of == c
        pos, chv = s_pos[m], s_q[m]
        dl = dst[m] - c * NPC
        t = dl // P
        w = wave_of_tile[t]
        key = (w * NCH + chv) * NTILES + t
        o = np.argsort(key, kind="stable")
        pos, chv, t, dl, w, key = pos[o], chv[o], t[o], dl[o], w[o], key[o]
        cnt3[c] = np.bincount(key, minlength=NW * NCH * NTILES).reshape(
            NW, NCH, NTILES
        )
        per_core_raw.append((pos, chv, t, dl - t * P, w, key))

    # within-call offsets of each tile segment (per core), call sizes
    off_tc = np.zeros((N_CORES, NTILES, NCH), np.int64)
    cnt_tc = np.zeros((N_CORES, NTILES, NCH), np.int64)
    cnt_call = np.zeros((N_CORES, NW, NCH), np.int64)
    for c in range(N_CORES):
        for wi, (q, ts) in enumerate(waves):
            for ch in range(NCH):
                acc = 0
                for t in ts:
                    off_tc[c, t, ch] = acc
                    cnt_tc[c, t, ch] = cnt3[c, wi, ch, t]
                    acc += cnt3[c, wi, ch, t]
                cnt_call[c, wi, ch] = acc
    call_nb = np.maximum((cnt_call.max(axis=0) + P - 1) // P, 1)  # [NW, NCH]

    # static per-(tile, chunk) block span (union over cores)
    blk0 = np.zeros((NTILES, NCH), np.int64)
    blk1 = np.zeros((NTILES, NCH), np.int64)
    any_cnt = cnt_tc.max(axis=0) > 0                # [NTILES, NCH]
    for t in range(NTILES):
        for ch in range(NCH):
            if not any_cnt[t, ch]:
                continue
            s = off_tc[:, t, ch]
            e = s + cnt_tc[:, t, ch]
            act = cnt_tc[:, t, ch] > 0
            blk0[t, ch] = (s[act] // P).min()
            blk1[t, ch] = ((e[act] + P - 1) // P).max()

    # sel order: (wave, tile, chunk, block)
    sel_base = np.zeros((NTILES, NCH), np.int64)
    wave_sel_start = np.zeros(NW + 1, np.int64)
    g = 0
    for wi, (q, ts) in enumerate(waves):
        wave_sel_start[wi] = g
        for t in ts:
            for ch in range(NCH):
                if not any_cnt[t, ch]:
                    continue
                sel_base[t, ch] = g
                g += int(blk1[t, ch] - blk0[t, ch])
    wave_sel_start[NW] = g
    NB_SEL = g

    # idx slot order per chunk: (wave, block)
    chunk_start = np.zeros((NCH, NW), np.int64)
    S_ch = [0] * NCH
    for ch in range(NCH):
        acc = 0
        for wi in range(NW):
            chunk_start[ch, wi] = acc
            acc += int(call_nb[wi, ch]) * P
        S_ch[ch] = acc

    idx16 = [np.zeros((N_CORES, P, S_ch[ch] // 16), np.int16) for ch in range(NCH)]
    sel = np.zeros((N_CORES, P, NB_SEL, P), SEL_NP)

    for c in range(N_CORES):
        pos, chv, t, dd, w, key = per_core_raw[c]
        flat = np.bincount(key, minlength=NW * NCH * NTILES)
        starts = np.cumsum(flat) - flat
        i = np.arange(len(pos)) - starts[key]       # seq within (w, ch, t)
        slot_in_call = off_tc[c, t, chv] + i
        j = slot_in_call // P
        lane = slot_in_call % P
        selpos = sel_base[t, chv] + (j - blk0[t, chv])
        sel[c][lane, selpos, dd] = 1.0
        chunk_slot = chunk_start[chv, w] + slot_in_call
        for ch in range(NCH):
            mm = chv == ch
            # trailing pad slots of each call are -1: desc-gen skips them
            # (count passed per-core via num_idxs_reg)
            slots_arr = np.full(S_ch[ch], -1, np.int16)
            slots_arr[chunk_slot[mm]] = pos[mm].astype(np.int16)
            idx16[ch][c] = np.tile(slots_arr.reshape(-1, 16).T, (8, 1))

    meta = {
        "waves": waves,
        "call_nb": call_nb,
        "blk0": blk0,
        "blk1": blk1,
        "any_cnt": any_cnt,
        "sel_base": sel_base,
        "wave_sel_start": wave_sel_start,
        "NB_SEL": NB_SEL,
        "chunk_start": chunk_start,
        "S_ch": S_ch,
        "cnt_call": cnt_call,
    }
    return dinv, idx16, sel, meta


def _build_program(meta):
    waves = meta["waves"]
    call_nb = meta["call_nb"]
    blk0 = meta["blk0"]
    blk1 = meta["blk1"]
    any_cnt = meta["any_cnt"]
    sel_base = meta["sel_base"]
    wave_sel_start = meta["wave_sel_start"]
    NB_SEL = meta["NB_SEL"]
    chunk_start = meta["chunk_start"]
    S_ch = meta["S_ch"]
    NW = len(waves)

    MSG_NB = int(call_nb.max())
    SEL_NB = int(np.diff(wave_sel_start).max())

    nc = bacc.Bacc(
        "TRN2", target_bir_lowering=False, debug=False, num_devices=N_CORES,
        num_swdge_queues=NQUEUES, dynamic_dma_scratch_size=DMA_SCRATCH,
    )

    xT_io = nc.dram_tensor("xT", [F0, NPC], dt.bfloat16, kind="ExternalInput").ap()
    dinv_io = nc.dram_tensor("dinvT", [P, NTILES], dt.float32, kind="ExternalInput").ap()
    rdinv_io = nc.dram_tensor("rdinvR", [65, 6272], dt.bfloat16, kind="ExternalInput").ap()
    w1_io = nc.dram_tensor("W1sb", [P, F0], dt.bfloat16, kind="ExternalInput").ap()
    w2_io = nc.dram_tensor("W2sb", [P, 64], dt.bfloat16, kind="ExternalInput").ap()
    w3_io = nc.dram_tensor("W3sb", [64, 32], dt.bfloat16, kind="ExternalInput").ap()
    brow_ios = [
        nc.dram_tensor(f"b{l+1}row", [65, NCOLS[l]], dt.bfloat16, kind="ExternalInput").ap()
        for l in range(3)
    ]
    identb_io = nc.dram_tensor("identB", [P, P], dt.float32, kind="ExternalInput").ap()
    identf8_io = nc.dram_tensor("identF8", [P, P], SEL_DT, kind="ExternalInput").ap()
    identbb_io = nc.dram_tensor("identBB", [P, P], dt.bfloat16, kind="ExternalInput").ap()
    idx_ios = [
        nc.dram_tensor(f"idx{ch}", [P, S_ch[ch] // 16], dt.int16, kind="ExternalInput").ap()
        for ch in range(NCH)
    ]
    sel_io = nc.dram_tensor("sel", [P, NB_SEL * P], SEL_DT, kind="ExternalInput").ap()
    gcnt_io = nc.dram_tensor("gcnt", [1, NW * NCH], dt.int32, kind="ExternalInput").ap()
    out_io = nc.dram_tensor("out3", [NPC, 32], dt.float32, kind="ExternalOutput").ap()

    with tile.TileContext(nc) as tc:
        with (
            tc.tile_pool(name="const", bufs=1) as constp,
            tc.tile_pool(name="hskeep", bufs=1) as hkp,
            tc.tile_pool(name="xT", bufs=4) as xTp,
            tc.tile_pool(name="idxw", bufs=6) as idxp,
            tc.tile_pool(name="msgs", bufs=MSG_BUFS) as msgp,
            tc.tile_pool(name="sel", bufs=SEL_BUFS) as selp,
            tc.tile_pool(name="outt", bufs=7) as outtp,
            tc.tile_pool(name="att", bufs=3) as attp,
            tc.tile_pool(name="psum_a", bufs=4, space="PSUM") as psap,
            tc.tile_pool(name="psum_t", bufs=2, space="PSUM") as pstp,
            tc.tile_pool(name="psum_d", bufs=2, space="PSUM") as psdp,
            tc.tile_pool(name="dram", bufs=1, space="DRAM") as dramp,
        ):
            # ---- constants ----
            dinv_sb = constp.tile([P, NTILES], dt.float32)
            nc.sync.dma_start(dinv_sb[:], dinv_io[:])
            # two-region layout: tiles 0-48 on partition 0, 49-97 on
            # partition 64 (matmul lhsT base partition must be 0/32/64)
            rdinv_sb = constp.tile([65, 6272], dt.bfloat16)
            nc.sync.dma_start(rdinv_sb[0:1, :], rdinv_io[0:1, :])
            nc.sync.dma_start(rdinv_sb[64:65, :], rdinv_io[64:65, :])
            w1_sb = constp.tile([P, F0], dt.bfloat16)
            nc.sync.dma_start(w1_sb[:], w1_io[:])
            w2_sb = constp.tile([P, 64], dt.bfloat16)
            nc.sync.dma_start(w2_sb[:], w2_io[:])
            w3_sb = constp.tile([64, 32], dt.bfloat16)
            nc.sync.dma_start(w3_sb[:], w3_io[:])
            brow_sb = []
            for l in range(3):
                # bias row duplicated at partitions 0 and 64 to match the
                # two-region rdinv lhsT base partition
                bt = constp.tile([65, NCOLS[l]], dt.bfloat16, tag=f"brow{l}")
                nc.sync.dma_start(bt[0:1, :], brow_ios[l][0:1, :])
                nc.sync.dma_start(bt[64:65, :], brow_ios[l][64:65, :])
                brow_sb.append(bt)
            identb = constp.tile([P, P], dt.float32)
            nc.sync.dma_start(identb[:], identb_io[:])
            identf8 = constp.tile([P, P], SEL_DT)
            nc.sync.dma_start(identf8[:], identf8_io[:])
            identbb = constp.tile([P, P], dt.bfloat16)
            nc.sync.dma_start(identbb[:], identbb_io[:])
            gcnt_sb = constp.tile([1, NW * NCH], dt.int32)
            nc.sync.dma_start(gcnt_sb[:], gcnt_io[:])
            gcnt_reg = nc.alloc_register(mybir.EngineType.Pool, "gcnt_reg")

            hk = hkp.tile([P, NTILES * P], dt.bfloat16, tag="hk")

            # one-time zero of the msg buffers: slots skipped by the gather
            # (trailing -1 idxs) must never expose uninitialized SBUF (NaN
            # bit patterns would poison the 0-weighted sel matmuls)
            for _ in range(MSG_BUFS):
                mz = msgp.tile([P, MSG_NB, FW], dt.bfloat16, tag="msg")
                nc.vector.memset(mz[:], 0.0)

            bounces = [
                [
                    dramp.tile([SQR[q], FW], dt.bfloat16, tag=f"bnc{l}_{q}",
                               name=f"bnc{l}_{q}")
                    for q in range(4)
                ]
                for l in range(3)
            ]
            tbls = [
                [
                    dramp.tile([CH_ROWS[q], FW], dt.bfloat16, tag=f"tbl{l}_{q}",
                               name=f"tbl{l}_{q}",
                               addr_space="Shared" if TBL_SHARED else "Local")
                    for q in range(4)
                ]
                for l in range(3)
            ]

            def emit_ag(l, q):
                nc.gpsimd.collective_compute(
                    "AllGather", mybir.AluOpType.bypass,
                    replica_groups=[list(range(N_CORES))],
                    ins=[bounces[l][q].opt()], outs=[tbls[l][q].opt()],
                )

            # ---- dense0: hs1 = dinv * (x @ W1); grouped reads, quarter AGs ----
            nk = F0 // P
            DG = 8
            for t0 in range(0, NTILES, DG):
                nt = min(DG, NTILES - t0)
                ncol = min(NPC, (t0 + nt) * P) - t0 * P
                xts = []
                for k in range(nk):
                    xt = xTp.tile([P, DG * P], dt.bfloat16, tag="xt")
                    nc.sync.dma_start(
                        xt[:, :ncol], xT_io[k * P:(k + 1) * P, t0 * P:t0 * P + ncol]
                    )
                    xts.append(xt)
                for g in range(nt):
                    t = t0 + g
                    r = _rows(t)
                    c0 = t * P
                    q = _quarter_of_tile(t)
                    pd = psdp.tile([P, P], dt.float32, space="PSUM", tag="pd")
                    for k in range(nk):
                        nc.tensor.matmul(
                            out=pd[:r, :], lhsT=xts[k][:, g * P:g * P + r],
                            rhs=w1_sb[:, k * P:(k + 1) * P],
                            start=(k == 0), stop=(k == nk - 1),
                        )
                    nc.vector.tensor_scalar_mul(
                        hk[:r, c0:c0 + P], pd[:r, :], dinv_sb[:r, t:t + 1]
                    )
                    # bounce writes go on the Activation HWDGE queues so the
                    # AG triggers' completion-counter waits aren't polluted by
                    # sel/idx/gather traffic on the SP queues
                    nc.scalar.dma_start(
                        bounces[0][q][c0 - QSTART[q]:c0 - QSTART[q] + r, :],
                        hk[:r, c0:c0 + P],
                    )
                    if t == QT[q + 1] - 1:
                        emit_ag(0, q)

            # ---- fused aggregation + next dense, per layer ----
            # AG(0, q) triggers are interleaved into the layer-0 gather
            # emission (each before the first gather needing chunk q);
            # AG(l+1, q) triggers are emitted TRIG_LAG waves after quarter
            # q's aggregation waves are emitted, so the (in-order) gpsimd
            # queue doesn't stall waiting for its bounces.
            last_wave_of_q = {q: max(wi for wi, (qq, _) in enumerate(waves) if qq == q)
                              for q in range(4)}

            for l in range(3):
                ncols = NCOLS[l]
                fout = NCOLS[l + 1] if l < 2 else 0
                w_next = (w2_sb, w3_sb)[l] if l < 2 else None

                msg_tiles = {}   # (w, ch) -> tile
                sel_tiles = {}   # w -> tile

                def ensure_sel(w):
                    if w >= NW or w in sel_tiles:
                        return
                    # prefetch the wave's sel slice ahead of its aggregation
                    sb0 = int(wave_sel_start[w])
                    sbn = int(wave_sel_start[w + 1]) - sb0
                    selt = selp.tile([P, SEL_NB, P], SEL_DT, tag="sel")
                    sel_tiles[w] = selt
                    nc.sync.dma_start(
                        selt[:, :sbn, :], sel_io[:, sb0 * P:(sb0 + sbn) * P]
                    )

                def emit_gather(w, ch, l=l):
                    ensure_sel(w)
                    nb = int(call_nb[w, ch])
                    mt = msgp.tile([P, MSG_NB, FW], dt.bfloat16, tag="msg")
                    msg_tiles[(w, ch)] = mt
                    s0 = int(chunk_start[ch, w])
                    iw = idxp.tile([P, MSG_NB * 8], dt.int16, tag="idx")
                    nc.sync.dma_start(
                        iw[:, :nb * 8], idx_ios[ch][:, s0 // 16:(s0 + nb * P) // 16]
                    )
                    assert nb <= MAXB, "per-core count registers assume one call"
                    nc.gpsimd.reg_load(
                        gcnt_reg, gcnt_sb[0:1, w * NCH + ch:w * NCH + ch + 1]
                    )
                    nc.gpsimd.dma_gather(
                        out_ap=mt[:, 0:nb, :],
                        in_ap=tbls[l][ch][:, :],
                        idxs_ap=iw[:, 0:nb * 8],
                        num_idxs=nb * P, num_idxs_reg=gcnt_reg,
                        elem_size=FW, elem_step=FW,
                        single_packet=False,
                        queue_num=ch,
                    )

                def emit_agg(w, l=l, ncols=ncols, fout=fout, w_next=w_next):
                    q, wtiles = waves[w]
                    sb0 = int(wave_sel_start[w])
                    # pass 1: aggregation matmuls + relu for every tile, so
                    # the PE never stalls mid-wave on the scalar engine
                    outts = {}
                    for t in wtiles:
                        r = _rows(t)
                        c0 = t * P
                        pa = psap.tile([P, ncols], dt.float32, space="PSUM", tag="pa")
                        # self-loop rows from the local hs tile (PSUM init)
                        nc.tensor.matmul(
                            out=pa[:r, :], lhsT=identf8[:r, :r],
                            rhs=hk[:r, c0:c0 + ncols],
                            start=True, stop=False,
                        )
                        mms = []
                        for ch in range(NCH):
                            if not any_cnt[t, ch]:
                                continue
                            for j in range(int(blk0[t, ch]), int(blk1[t, ch])):
                                sp = int(sel_base[t, ch]) + j - int(blk0[t, ch])
                                mms.append((sp - sb0, ch, j))
                        # bias outer product: (1/dinv)[d] * b[f]; closes the
                        # accumulation group when a tile has no edge blocks
                        rp, rc = (0, t * P) if t < 49 else (64, (t - 49) * P)
                        nc.tensor.matmul(
                            out=pa[:r, :], lhsT=rdinv_sb[rp:rp + 1, rc:rc + r],
                            rhs=brow_sb[l][rp:rp + 1, :],
                            start=False, stop=(len(mms) == 0),
                        )
                        for i, (sj, ch, col) in enumerate(mms):
                            nc.tensor.matmul(
                                out=pa[:r, :], lhsT=sel_tiles[w][:, sj, :r],
                                rhs=msg_tiles[(w, ch)][:, col, :ncols],
                                start=False, stop=(i == len(mms) - 1),
                            )
                        if l < 2:
                            outt = outtp.tile([P, P], dt.bfloat16, tag="outt")
                            nc.scalar.activation(
                                outt[:r, :ncols], pa[:r, :],
                                mybir.ActivationFunctionType.Relu,
                                scale=dinv_sb[:r, t:t + 1],
                            )
                            outts[t] = outt
                        else:
                            o32 = outtp.tile([P, 32], dt.float32, tag="o32")
                            nc.scalar.activation(
                                o32[:r, :], pa[:r, :32],
                                mybir.ActivationFunctionType.Relu,
                                scale=dinv_sb[:r, t:t + 1],
                            )
                            nc.sync.dma_start(out_io[c0:c0 + r, :], o32[:r, :])
                    if l == 2:
                        return
                    # pass 2: transpose + next-layer dense per tile
                    for t in wtiles:
                        r = _rows(t)
                        c0 = t * P
                        tq = _quarter_of_tile(t)
                        outt = outts[t]
                        pst = pstp.tile([P, P], dt.bfloat16, space="PSUM", tag="pt")
                        nc.tensor.transpose(
                            out=pst[:ncols, :r], in_=outt[:r, :ncols],
                            identity=identbb[:r, :r],
                        )
                        att = attp.tile([P, P], dt.bfloat16, tag="att")
                        nc.scalar.copy(att[:ncols, :r], pst[:ncols, :r])
                        pd = psdp.tile([P, P], dt.float32, space="PSUM", tag="pd")
                        nc.tensor.matmul(
                            out=pd[:r, :fout], lhsT=att[:ncols, :r], rhs=w_next[:, :],
                            start=True, stop=True,
                        )
                        nc.vector.tensor_scalar_mul(
                            hk[:r, c0:c0 + fout], pd[:r, :fout], dinv_sb[:r, t:t + 1]
                        )
                        nc.scalar.dma_start(
                            bounces[l + 1][tq][c0 - QSTART[tq]:c0 - QSTART[tq] + r, :],
                            hk[:r, c0:c0 + P],
                        )

                # AG triggers for the NEXT layer, placed TRIG_LAG waves after
                # the quarter's last aggregation wave (or at the very end).
                trig_at = {}
                tail_qs = []
                if l < 2:
                    for q in range(4):
                        wpos = last_wave_of_q[q] + TRIG_LAG
                        if wpos < NW:
                            trig_at.setdefault(wpos, []).append(q)
                        else:
                            tail_qs.append(q)

                for w in range(NW):
                    if w == 0:
                        # chunk-major across the first two waves: each chunk's
                        # gathers grouped so a not-yet-ready chunk never
                        # blocks a ready one
                        for ch in range(3):
                            for wl in range(min(3, NW)):
                                emit_gather(wl, ch)
                    else:
                        if w + 1 < NW:
                            for ch in range(3):
                                emit_gather(w + 1, ch)
                    emit_gather(w, 3)
                    emit_agg(w)
                    ensure_sel(w + 2)
                    for q in trig_at.get(w, ()):
                        emit_ag(l + 1, q)
                for q in tail_qs:
                    emit_ag(l + 1, q)

    nc.compile()
    return nc


def _pack_inputs(x, dinv, W1, b1, W2, b2, W3, b3, idx16, sel, cnt_call):
    w1sb = np.zeros((P, F0), np.float32)
    for k in range(F0 // P):
        w1sb[:, k * P:(k + 1) * P] = W1[k * P:(k + 1) * P, :]
    identb = np.eye(P, dtype=np.float32)

    in_maps = []
    for c in range(N_CORES):
        lo = c * NPC
        xs = x[lo:lo + NPC].astype(np.float32)
        dv = dinv[lo:lo + NPC]
        dvt = np.ones((P, NTILES), np.float32)
        rdv = np.zeros((65, 6272), np.float32)
        for t in range(NTILES):
            r = _rows(t)
            dvt[:r, t] = dv[t * P:t * P + r]
            rp, rc = (0, t * P) if t < 49 else (64, (t - 49) * P)
            rdv[rp, rc:rc + r] = 1.0 / dv[t * P:t * P + r]
        in_maps.append({
            "xT": np.ascontiguousarray(xs.T).astype(BF),
            "dinvT": dvt,
            "rdinvR": rdv.astype(BF),
            "W1sb": w1sb.astype(BF),
            "W2sb": W2.astype(BF),
            "W3sb": W3.astype(BF),
            "b1row": np.tile(b1[None, :], (65, 1)).astype(BF),
            "b2row": np.tile(b2[None, :], (65, 1)).astype(BF),
            "b3row": np.tile(b3[None, :], (65, 1)).astype(BF),
            "identB": identb.astype(np.float32),
            "identBB": identb.astype(BF),
            "identF8": identb.astype(SEL_NP),
            **{f"idx{ch}": idx16[ch][c] for ch in range(NCH)},
            "sel": sel[c].reshape(P, -1),
            "gcnt": cnt_call[c].reshape(1, -1).astype(np.int32),
        })
    return in_maps


_TRACE = [False]          # set by test harness to request a profiled run
_LAST_RESULT = [None]     # BassKernelResults of the last run (for profiling)


def kernel(x, edge_index, batch, W1, b1, W2, b2, W3, b3, Wfc, bfc):
    x = np.asarray(x)
    edge_index = np.asarray(edge_index)
    batch = np.asarray(batch)
    W1, b1 = np.asarray(W1), np.asarray(b1)
    W2, b2 = np.asarray(W2), np.asarray(b2)
    W3, b3 = np.asarray(W3), np.asarray(b3)
    Wfc, bfc = np.asarray(Wfc), np.asarray(bfc)

    dinv, idx16, sel, meta = _host_prep(edge_index.astype(np.int64))
    nc = _build_program(meta)
    in_maps = _pack_inputs(x, dinv, W1, b1, W2, b2, W3, b3, idx16, sel,
                           meta["cnt_call"])
    res = run_bass_kernel_spmd(
        nc, in_maps, core_ids=list(range(N_CORES)), trace=_TRACE[0]
    )
    _LAST_RESULT[0] = res

    h3 = np.concatenate([res.results[c]["out3"] for c in range(N_CORES)], axis=0)

    # host epilogue: segment max pool + FC + log_softmax (float64 for stability)
    pooled = np.full((N_GRAPHS, 32), -np.inf, np.float64)
    bnd = np.searchsorted(batch, np.arange(N_GRAPHS + 1))
    for g in range(N_GRAPHS):
        if bnd[g + 1] > bnd[g]:
            pooled[g] = h3[bnd[g]:bnd[g + 1]].max(axis=0)
    logits = pooled @ Wfc.astype(np.float64) + bfc.astype(np.float64)
    m = logits.max(axis=1, keepdims=True)
    lse = m + np.log(np.exp(logits - m).sum(axis=1, keepdims=True))
    return (logits - lse).astype(np.float32)
